# revision 3
# baseline (speedup 1.0000x reference)
"""DA-RNN Trainium2 Bass kernel, v2.

Key changes vs baseline:
- "x2-state" LSTM algebra: store D=2c and H2=2h. All four gates go through
  ONE tanh form: t = tanh(0.5*pre), with sigmoid(z)=(tanh(z/2)+1)/2 folded
  into the elementwise tail:
      m1 = (t_f+1)*D ; m2 = (t_i+1)*t_g ; D' = 0.5*m1 + m2
      tanh(c') = tanh(0.5*D') ; H2' = (t_o+1)*tanh(c')
  Weights consuming h are pre-scaled x0.5 host-side; the g-gate rows are
  pre-scaled x2 so tanh(0.5*pre) == tanh(g).
- reciprocal_approx_fast for softmax denominators (was 945ns InstReciprocal).
- softmax 1/s partition-broadcast matmul in float32r (was fp32 LOW_HIGH pair).
- gate order (f,i,g,o); cell tail uses scalar_tensor_tensor 3-op form.
- decoder per-gate biases via ACT bias APs (drops 12 bias matmuls).
"""

import numpy as np
import ml_dtypes

import concourse.bacc as bacc
import concourse.tile as tile
import concourse.mybir as mybir
from concourse.bass_utils import run_bass_kernel_spmd

from concourse.dve_ops import RECIP_APPROX_FAST_CONSTS, RECIPROCAL_APPROX_FAST

F32 = mybir.dt.float32
F32R = mybir.dt.float32r
BF16 = mybir.dt.bfloat16
AF = mybir.ActivationFunctionType
OP = mybir.AluOpType

L, NOUT, F, B, H = 50, 3, 64, 2048, 128
NC = 8
BPC = B // NC          # 256 batch per core
CH = 2                 # encoder chains (batch halves of 128)
BH = BPC // CH         # 128

bf16 = ml_dtypes.bfloat16

# gate order used on-chip: (f, i, g, o). PyTorch weights are (i, f, g, o).
GATE_PERM = [1, 0, 2, 3]  # source block (in pytorch order) for f,i,g,o


def _gate_rows(w, g):
    src = GATE_PERM[g]
    return w[src * H:(src + 1) * H]


# per-gate scale applied to ALL terms of the pre-activation (x2 on g so the
# single tanh(0.5*pre) covers both sigmoid- and tanh-gates)
GSC = [1.0, 1.0, 2.0, 1.0]


def prep_inputs(inputs):
    f32 = np.float32
    x = np.asarray(inputs["x"], f32)            # [B, L, F]

    shared = {}
    # encoder attention dense: attn_w [L, F+H, F]
    aw = np.asarray(inputs["attn_w"], f32)
    shared["w1a"] = np.ascontiguousarray(aw[:, :F, :].transpose(1, 0, 2)).astype(bf16)
    # h-consumer: x0.5 (h = H2/2)
    shared["w2a"] = np.ascontiguousarray(
        0.5 * aw[:, F:, :].transpose(1, 0, 2)).astype(bf16)
    shared["battn"] = np.ascontiguousarray(np.asarray(inputs["attn_b"], f32).T)  # [64, L]

    # encoder LSTM: wih65 rows 0-63 = Wih.T (gate-scaled), row 64 = bias
    wih = np.asarray(inputs["enc_Wih"], f32)    # [4H, F]
    whh = np.asarray(inputs["enc_Whh"], f32)    # [4H, H]
    bias = np.asarray(inputs["enc_bih"], f32) + np.asarray(inputs["enc_bhh"], f32)
    wih65 = np.zeros((F + 1, 4, H), f32)
    whhT = np.zeros((H, 4, H), f32)
    for g in range(4):
        wih65[:F, g, :] = GSC[g] * _gate_rows(wih, g).T
        wih65[F, g, :] = GSC[g] * _gate_rows(bias[:, None], g)[:, 0]
        whhT[:, g, :] = (0.5 * GSC[g]) * _gate_rows(whh, g).T  # consumes H2
    shared["wih65"] = wih65.astype(bf16)
    shared["whhT"] = whhT.astype(bf16)

    # decoder attention (consume H2 enc states and H2de): x0.5
    ddw = np.asarray(inputs["dd_w"], f32)       # [NOUT, 2H, H]
    shared["ddw1"] = np.ascontiguousarray(
        0.5 * ddw[:, :H, :].transpose(1, 0, 2)).astype(bf16)   # [128, NOUT, 128]
    shared["ddw2"] = np.ascontiguousarray(
        0.5 * ddw[:, H:, :].transpose(1, 0, 2)).astype(bf16)
    shared["ddb"] = np.ascontiguousarray(np.asarray(inputs["dd_b"], f32).T)  # [128, NOUT]
    dlw = np.asarray(inputs["dl_w"], f32)[:, :, 0].T                         # [128, NOUT]
    shared["dlw"] = np.ascontiguousarray(
        np.repeat(dlw[:, :, None], 32, axis=2)).astype(bf16)                 # [128, NOUT, 32]
    shared["dlb"] = np.asarray(inputs["dl_b"], f32)[:, 0]                    # [NOUT]

    # decoder LSTM: dec_in = [ctx, dec_out]; dec_out == h_de merges with Whh.
    # ctx arrives as ctx2=2*ctx (x0.5), h_de as H2de (x0.5). Gate-scaled.
    dwih = np.asarray(inputs["dec_Wih"], f32)   # [4H, 2H]
    dwhh = np.asarray(inputs["dec_Whh"], f32)   # [4H, H]
    dbias = np.asarray(inputs["dec_bih"], f32) + np.asarray(inputs["dec_bhh"], f32)
    wdic = np.zeros((H, 4, H), f32)
    wdoh = np.zeros((H, 4, H), f32)
    dbias4 = np.zeros((H, 4), f32)
    for g in range(4):
        wdic[:, g, :] = (0.5 * GSC[g]) * _gate_rows(dwih[:, :H], g).T
        wdoh[:, g, :] = (0.5 * GSC[g]) * (
            _gate_rows(dwih[:, H:], g) + _gate_rows(dwhh, g)).T
        # ACT bias: tanh(0.5*pre + bias_ap) must equal tanh(0.5*(raw + b))
        # for f,i,o (bias_ap = b/2) and tanh(raw + b) for g (bias_ap = b).
        dbias4[:, g] = (0.5 * GSC[g]) * _gate_rows(dbias[:, None], g)[:, 0]
    shared["wdic"] = wdic.astype(bf16)
    shared["wdoh"] = wdoh.astype(bf16)
    shared["dbias4"] = dbias4

    # heads (consume H2de): x0.5
    shared["fcw"] = np.ascontiguousarray(
        0.5 * np.asarray(inputs["fc_w"], f32).transpose(1, 0, 2)).astype(bf16)
    shared["fcb"] = np.ascontiguousarray(np.asarray(inputs["fc_b"], f32).T)  # [64, NOUT]
    shared["outw"] = np.ascontiguousarray(
        np.asarray(inputs["out_w"], f32)[:, :, 0].T).astype(bf16)            # [64, NOUT]
    shared["outb"] = np.asarray(inputs["out_b"], f32)[:, 0]                  # [NOUT]

    per_core = []
    for c in range(NC):
        xc = x[c * BPC:(c + 1) * BPC]           # [256, L, F]
        xT = np.ascontiguousarray(xc.transpose(2, 1, 0)).astype(bf16)  # [64, L, 256]
        per_core.append({"xT": xT})
    return shared, per_core


def build_program():
    nc = bacc.Bacc("TRN2", target_bir_lowering=False, debug=False, num_devices=NC)

    dram = {}

    def din(name, shape, dt):
        dram[name] = nc.dram_tensor(name, shape, dt, kind="ExternalInput").ap()
        return dram[name]

    din("xT", (F, L, BPC), BF16)
    din("w1a", (F, L, F), BF16)
    din("w2a", (H, L, F), BF16)
    din("battn", (F, L), F32)
    din("wih65", (F + 1, 4, H), BF16)
    din("whhT", (H, 4, H), BF16)
    din("ddw1", (H, NOUT, H), BF16)
    din("ddw2", (H, NOUT, H), BF16)
    din("ddb", (H, NOUT), F32)
    din("dlw", (H, NOUT, 32), BF16)
    din("wdic", (H, 4, H), BF16)
    din("wdoh", (H, 4, H), BF16)
    din("dbias4", (H, 4), F32)
    din("fcw", (H, NOUT, F), BF16)
    din("fcb", (F, NOUT), F32)
    din("outw", (F, NOUT), BF16)
    y_out = nc.dram_tensor("y", (NOUT, BPC), F32, kind="ExternalOutput").ap()
    dlb_sc = build_program.scalars["dlb"]
    outb_sc = build_program.scalars["outb"]

    with tile.TileContext(nc) as tc:
        _body(nc, tc, dram, y_out, dlb_sc, outb_sc)
    nc.compile()
    return nc, list(dram.keys())


build_program.scalars = {"dlb": [0.0] * NOUT, "outb": [0.0] * NOUT}


def _body(nc, tc, dram, y_out, dlb_sc, outb_sc):
    import contextlib
    ctx = contextlib.ExitStack()
    with ctx:
        singles = ctx.enter_context(tc.tile_pool(name="singles", bufs=1))

        def load(name, shape, dt):
            t = singles.tile(list(shape), dt, tag=name)
            nc.sync.dma_start(out=t, in_=dram[name])
            return t

        xT = load("xT", (F, L, BPC), BF16)
        w1a = load("w1a", (F, L, F), BF16)
        w2a = load("w2a", (H, L, F), BF16)
        battn = load("battn", (F, L), F32)
        wih65 = load("wih65", (F + 1, 4, H), BF16)
        whhT = load("whhT", (H, 4, H), BF16)
        ddw1 = load("ddw1", (H, NOUT, H), BF16)
        ddw2 = load("ddw2", (H, NOUT, H), BF16)
        ddb = load("ddb", (H, NOUT), F32)
        dlw = load("dlw", (H, NOUT, 32), BF16)
        wdic = load("wdic", (H, 4, H), BF16)
        wdoh = load("wdoh", (H, 4, H), BF16)
        dbias4 = load("dbias4", (H, 4), F32)
        fcw = load("fcw", (H, NOUT, F), BF16)
        fcb = load("fcb", (F, NOUT), F32)
        outw = load("outw", (F, NOUT), BF16)

        encT = singles.tile([H, L, BPC], BF16, tag="encT")      # H2 states
        encBh = singles.tile([BH, CH, L, H], BF16, tag="encBh")
        encB = singles.tile([BH, CH, H, L], BF16, tag="encB")
        xin65 = singles.tile([F + 1, CH, BH], BF16, tag="xin65")
        cg = singles.tile([H, CH, BH], F32, tag="cg")           # D = 2c
        ones64 = singles.tile([F, 1], BF16, tag="ones64")
        ones1b = singles.tile([1, F], BF16, tag="ones1b")
        hdeT = singles.tile([H, BPC], BF16, tag="hdeT")         # H2de
        dcg = singles.tile([H, BPC], F32, tag="dcg")            # decoder D
        ySB = singles.tile([1, NOUT, BPC], F32, tag="ySB")

        dlbT = singles.tile([64, NOUT], F32, tag="dlbT")
        outbT = singles.tile([1, NOUT], F32, tag="outbT")
        for i in range(NOUT):
            nc.vector.memset(dlbT[:, i:i + 1], float(dlb_sc[i]))
            nc.vector.memset(outbT[:, i:i + 1], float(outb_sc[i]) * 0.5)

        nc.vector.memset(xin65[F:F + 1, :, :], 1.0)
        nc.vector.memset(cg, 0.0)
        nc.vector.memset(ones64, 1.0)
        nc.vector.memset(ones1b, 1.0)
        nc.vector.memset(dcg, 0.0)

        STT = nc.vector.scalar_tensor_tensor
        CHK = 4  # decoder-attention l per chunk
        nchunks = (L + CHK - 1) // CHK

        # ================= encoder =================
        with tc.tile_pool(name="psE", bufs=2, space="PSUM") as psE, \
             tc.tile_pool(name="psS", bufs=2, space="PSUM") as psS, \
             tc.tile_pool(name="psB", bufs=2, space="PSUM") as psB, \
             tc.tile_pool(name="psG", bufs=2, space="PSUM") as psG, \
             tc.tile_pool(name="enc_sb", bufs=3) as sb:

            for t in range(L):
                for c in range(CH):
                    bs = slice(c * BH, (c + 1) * BH)
                    h_prev = encT[:, t - 1, bs] if t > 0 else None

                    # attention pre-act FIRST: w2a@h starts the serial
                    # chain and must not queue behind the whh gate matmuls
                    # (which also wait on h but aren't needed until ~2.5us
                    # later) on the in-order PE
                    pe = psE.tile([F, BH], F32, tag="pe")
                    nc.tensor.matmul(pe, w1a[:, t, :], xT[:, t, bs],
                                     start=True, stop=(t == 0))
                    if t > 0:
                        nc.tensor.matmul(pe, w2a[:, t, :], h_prev,
                                         start=False, stop=True)
                    pg = psG.tile([H, 4, BH], F32, tag="pg")
                    if t > 0:
                        for g in range(4):
                            nc.tensor.matmul(pg[:, g, :], whhT[:, g, :], h_prev,
                                             start=True, stop=False)
                    eT = sb.tile([F, BH], BF16, tag="eT")
                    nc.scalar.activation(eT, pe, AF.Tanh, bias=battn[:, t:t + 1])
                    expE = sb.tile([F, BH], BF16, tag="expE")
                    nc.scalar.activation(expE, eT, AF.Exp)
                    # softmax denom across 64 feature partitions via PE
                    ps = psS.tile([1, BH], F32, tag="ps")
                    nc.tensor.matmul(ps, ones64, expE, start=True, stop=True)
                    xe = sb.tile([F, BH], BF16, tag="xe")
                    nc.vector.tensor_tensor(xe, expE, xT[:, t, bs], op=OP.mult)
                    rc = sb.tile([1, BH], BF16, tag="rc")
                    cst = RECIP_APPROX_FAST_CONSTS
                    nc.vector._custom_dve(RECIPROCAL_APPROX_FAST, out=rc, in0=ps,
                                          s0=cst["s0"], s1=cst["s1"],
                                          imm2=cst["imm2"])
                    pb = psB.tile([F, BH], F32, tag="pb")
                    nc.tensor.matmul(pb, ones1b, rc, start=True, stop=True)
                    # xin = softmax(e) * x_t (row 64 stays 1 for bias)
                    nc.vector.tensor_tensor(xin65[:F, c, :], xe, pb, op=OP.mult)

                    # gates += Wih65 @ [xin; 1]
                    for g in range(4):
                        nc.tensor.matmul(pg[:, g, :], wih65[:, g, :], xin65[:, c, :],
                                         start=(t == 0), stop=True)

                    # one tanh family: t_x = tanh(0.5*pre)
                    tfig = sb.tile([H, 3, BH], BF16, tag="tfig")
                    nc.scalar.activation(tfig, pg[:, 0:3, :], AF.Tanh, scale=0.5)
                    t_o = sb.tile([H, BH], BF16, tag="t_o")
                    nc.scalar.activation(t_o, pg[:, 3, :], AF.Tanh, scale=0.5)
                    # D' = 0.5*(t_f+1)*D + (t_i+1)*t_g ; H2 = (t_o+1)*tanh(D'/2)
                    m1 = sb.tile([H, BH], F32, tag="m1")
                    STT(m1, tfig[:, 0, :], 1.0, cg[:, c, :],
                        op0=OP.add, op1=OP.mult)
                    m2 = sb.tile([H, BH], F32, tag="m2")
                    STT(m2, tfig[:, 1, :], 1.0, tfig[:, 2, :],
                        op0=OP.add, op1=OP.mult)
                    STT(cg[:, c, :], m1, 0.5, m2, op0=OP.mult, op1=OP.add)
                    tcn = sb.tile([H, BH], BF16, tag="tcn")
                    nc.scalar.activation(tcn, cg[:, c, :], AF.Tanh, scale=0.5)
                    STT(encT[:, t, bs], t_o, 1.0, tcn, op0=OP.add, op1=OP.mult)
                    # batch-major copy for decoder context sums
                    nc.sync.dma_start_transpose(encBh[:, c, t, :], encT[:, t, bs])


        # ================= decoder =================
        nc.vector.tensor_copy(encB, encBh.rearrange("b c l h -> b c h l"))
        for i in range(NOUT):
            with tc.tile_pool(name="psDD", bufs=2, space="PSUM") as psDD, \
                 tc.tile_pool(name="psL", bufs=2, space="PSUM") as psL, \
                 tc.tile_pool(name="dec_sb", bufs=3) as sb, \
                 tc.tile_pool(name="ctx_sb", bufs=1) as csb:
                logitsL = sb.tile([64, BPC], F32, tag="logitsL")
                nc.vector.memset(logitsL, 0.0)
                for k in range(nchunks):
                    nl = min(CHK, L - k * CHK)
                    pdd = psDD.tile([H, CHK, BPC], F32, tag="pdd")
                    for j0 in range(0, nl, 2):
                        j1 = min(j0 + 2, nl)
                        nc.tensor.matmul(pdd[:, j0:j1, :], ddw1[:, i, :],
                                         encT[:, k * CHK + j0:k * CHK + j1, :],
                                         start=True, stop=(i == 0))
                        if i > 0:
                            nc.tensor.matmul(
                                pdd[:, j0:j1, :], ddw2[:, i, :],
                                hdeT[:, None, :].broadcast_to([H, j1 - j0, BPC]),
                                start=False, stop=True)
                    e2c = sb.tile([H, CHK, BPC], BF16, tag="e2c")
                    nc.scalar.activation(e2c[:, 0:nl, :], pdd[:, 0:nl, :],
                                         AF.Tanh, bias=ddb[:, i:i + 1])
                    pl = psL.tile([H, BPC], F32, tag="pl")
                    for j in range(nl):
                        nc.tensor.matmul(pl[32 * j:32 * (j + 1), :],
                                         dlw[:, i, :], e2c[:, j, :],
                                         start=True, stop=True,
                                         tile_position=(0, 32 * j))
                    if nl < CHK:
                        nc.vector.memset(pl[32 * nl:, :], 0.0)
                    lsc = sb.tile([H, BPC], F32, tag="lsc")
                    if k % 2 == 0:
                        nc.vector.tensor_copy(lsc, pl)
                    else:
                        nc.scalar.copy(lsc, pl)
                    nc.sync.dma_start(out=logitsL[k * CHK:k * CHK + nl, :],
                                      in_=lsc[0:32 * nl:32, :])
                expL = sb.tile([64, BPC], BF16, tag="expL")
                nc.vector.memset(expL, 0.0)
                nc.scalar.activation(expL[0:L, :], logitsL[0:L, :], AF.Exp,
                                     bias=dlbT[0:L, i:i + 1])
                expB = sb.tile([BH, CH, 64], BF16, tag="expB")
                for hh in range(CH):
                    nc.sync.dma_start_transpose(
                        expB[:, hh, :], expL[:, hh * BH:(hh + 1) * BH])
                z = sb.tile([BH, CH], F32, tag="z")
                nc.vector.tensor_reduce(z, expB[:, :, 0:L], axis=mybir.AxisListType.X,
                                        op=OP.add)
                rz = sb.tile([BH, CH], F32, tag="rz")
                nc.vector.reciprocal_approx_fast(out=rz, in_=z)

                # ctx2 = sum_l alpha * enc(H2)  (batch-major tree reduction)
                prod = csb.tile([BH, CH, H, L], BF16, tag="prod")
                nc.vector.tensor_tensor(
                    prod, encB,
                    expB[:, :, None, 0:L].broadcast_to([BH, CH, H, L]),
                    op=OP.mult)
                s25 = csb.tile([BH, CH, H, 25], BF16, tag="s25")
                nc.vector.tensor_tensor(s25, prod[:, :, :, 0:25],
                                        prod[:, :, :, 25:50], op=OP.add)
                s5 = sb.tile([BH, CH, H, 5], BF16, tag="s5")
                v25 = s25.rearrange("b c h (lo li) -> b c h lo li", lo=5)
                nc.vector.tensor_tensor(s5, v25[:, :, :, 0, :], v25[:, :, :, 1, :],
                                        op=OP.add)
                for j in (2, 3, 4):
                    nc.vector.tensor_tensor(s5, s5, v25[:, :, :, j, :], op=OP.add)
                ctxr = sb.tile([BH, CH, H], F32, tag="ctxr")
                nc.vector.tensor_tensor(ctxr, s5[:, :, :, 0], s5[:, :, :, 1],
                                        op=OP.add)
                for j in (2, 3, 4):
                    nc.vector.tensor_tensor(ctxr, ctxr, s5[:, :, :, j], op=OP.add)
                cn = sb.tile([BH, CH, H], BF16, tag="cn")
                for hh in range(CH):
                    nc.vector.tensor_scalar(cn[:, hh, :], ctxr[:, hh, :],
                                            rz[:, hh:hh + 1], None, op0=OP.mult)
                ctxT = sb.tile([H, BPC], BF16, tag="ctxT")
                for hh in range(CH):
                    nc.sync.dma_start_transpose(ctxT[:, hh * BH:(hh + 1) * BH],
                                                cn[:, hh, :])

            # decoder LSTM + heads (biases via ACT bias APs)
            with tc.tile_pool(name="psDG", bufs=1, space="PSUM") as psDG, \
                 tc.tile_pool(name="psY", bufs=1, space="PSUM") as psY, \
                 tc.tile_pool(name="dlstm_sb", bufs=2) as sb:
                pg = psDG.tile([H, 4, BPC], F32, tag="pdg")
                for g in range(4):
                    if i > 0:
                        nc.tensor.matmul(pg[:, g, :], wdoh[:, g, :], hdeT,
                                         start=True, stop=False)
                    nc.tensor.matmul(pg[:, g, :], wdic[:, g, :], ctxT,
                                     start=(i == 0), stop=True)
                tg4 = sb.tile([H, 4, BPC], BF16, tag="tg4")
                for g in range(4):
                    nc.scalar.activation(tg4[:, g, :], pg[:, g, :], AF.Tanh,
                                         scale=0.5, bias=dbias4[:, g:g + 1])
                m1 = sb.tile([H, BPC], F32, tag="dm1")
                STT(m1, tg4[:, 0, :], 1.0, dcg, op0=OP.add, op1=OP.mult)
                m2 = sb.tile([H, BPC], F32, tag="dm2")
                STT(m2, tg4[:, 1, :], 1.0, tg4[:, 2, :], op0=OP.add, op1=OP.mult)
                STT(dcg, m1, 0.5, m2, op0=OP.mult, op1=OP.add)
                tcn = sb.tile([H, BPC], BF16, tag="dtcn")
                nc.scalar.activation(tcn, dcg, AF.Tanh, scale=0.5)
                STT(hdeT, tg4[:, 3, :], 1.0, tcn, op0=OP.add, op1=OP.mult)

                py1 = psY.tile([F, BPC], F32, tag="py1")
                nc.tensor.matmul(py1, fcw[:, i, :], hdeT, start=True, stop=True)
                y1 = sb.tile([F, BPC], BF16, tag="y1")
                nc.scalar.activation(y1, py1, AF.Tanh, bias=fcb[:, i:i + 1])
                py2 = psY.tile([1, BPC], F32, tag="py2")
                nc.tensor.matmul(py2, outw[:, i:i + 1], y1, start=True, stop=True)
                yt = sb.tile([1, BPC], F32, tag="yt")
                nc.scalar.activation(yt, py2, AF.Tanh, scale=0.5,
                                     bias=outbT[:, i:i + 1])
                nc.vector.tensor_scalar(ySB[:, i, :], yt, 0.5, 0.5,
                                        op0=OP.mult, op1=OP.add)

        nc.sync.dma_start(out=y_out, in_=ySB)


_CACHE = {}


def kernel(**inputs):
    return _run(inputs, trace=False)[0]


def kernel_profiled(**inputs):
    return _run(inputs, trace=True)


def _run(inputs, trace=False):
    shared, per_core = prep_inputs(inputs)
    key = (float(shared["dlb"][0]), float(shared["outb"][0]),
           float(shared["dlb"][-1]), float(shared["outb"][-1]))
    if key not in _CACHE:
        build_program.scalars = {"dlb": shared["dlb"].tolist(),
                                 "outb": shared["outb"].tolist()}
        _CACHE[key] = build_program()
    nc, names = _CACHE[key]
    in_maps = []
    for c in range(NC):
        m = dict(shared)
        m.pop("dlb"), m.pop("outb")
        m.update(per_core[c])
        in_maps.append({k: np.ascontiguousarray(v) for k, v in m.items()})
    res = run_bass_kernel_spmd(nc, in_maps, core_ids=list(range(NC)), trace=trace)
    outs = [res.results[c]["y"].T for c in range(NC)]
    return np.concatenate(outs, axis=0).astype(np.float32), res


if __name__ == "__main__":
    pass
